# revision 8
# baseline (speedup 1.0000x reference)
"""DGP-RF embeddings kernel for 8 TRN2 NeuronCores (SPMD, full I/O).

Sharding: N=16384 rows split evenly, 2048 rows/core.  The segment softmax
is linear in disguise -- out[b] = segsum(emb_new*exp(p))[b]/segsum(exp(p))[b]
-- so each core returns partial numerator+denominator segment sums over its
rows (one-hot matmuls) and the host adds partials across cores and divides.

On-chip, activations are kept feature-major ([feat, n]) so every matmul
consumes natively-laid-out operands (weights [K,M] as lhsT, activations
[K,n] as rhs).  Big matmuls run in float32r (~fp22 multiply, fp32
accumulate, 1 cyc/row at N>=256).  LayerNorm exploits cos^2+sin^2=1:
var = 1/1024 - mu^2 exactly, so only the mean is needed (ones-lhsT matmul,
M=1).  1/sqrt(var+eps) is a Quake bitwise seed + 2 Newton steps on DVE over
a compact [8,128] batch, bounced through DRAM to partition-broadcast back
(SBUF APs cannot have stride-0 partitions; DRAM APs can).
cos/sin: custom DVE op add_range_wrap into [-pi,pi], then ACT Sin;
cos(z)=sin(pi/2-|wrap(z)|), keeping ACT on the trig table the whole main
loop (exp runs once at the end on spilled scores).
"""
import numpy as np

N_ROWS, B = 16384, 64
NMC, RF = 4, 512
D0, D1, D2 = 1024, 512, 256
NATT, DATT = 4, 32
NC = 8
RPC = N_ROWS // NC        # 2048 rows per core
NB = 4                    # n-blocks per core
NBS = RPC // NB           # 512 rows per block
P = 128
EPS = 1e-5
C_VAR = 1.0 / 1024.0 + EPS

_CACHE = {}


def _build(debug=False):
    import sys
    if "/opt/trn_rl_repo" not in sys.path:
        sys.path.insert(0, "/opt/trn_rl_repo")
    import concourse.mybir as mybir
    import concourse.tile as tile
    from concourse import bacc
    from concourse.masks import make_identity
    from contextlib import ExitStack

    dt = mybir.dt
    AF = mybir.ActivationFunctionType
    ALU = mybir.AluOpType
    f32 = dt.float32
    f32r = dt.float32r
    PI = float(np.pi)

    nc = bacc.Bacc()

    XT = nc.dram_tensor("XT", [D0, RPC], f32r, kind="ExternalInput")
    OH = nc.dram_tensor("OH", [RPC, B], f32r, kind="ExternalInput")
    OM1 = nc.dram_tensor("OM1", [NMC, D0, RF], f32r, kind="ExternalInput")
    OM2 = nc.dram_tensor("OM2", [NMC, D1, RF], f32r, kind="ExternalInput")
    W1T = nc.dram_tensor("W1T", [2 * RF, D1], f32r, kind="ExternalInput")
    W2T = nc.dram_tensor("W2T", [2 * RF, D2], f32r, kind="ExternalInput")
    WS4 = nc.dram_tensor("WS4", [D2, NATT], f32r, kind="ExternalInput")
    WSR = nc.dram_tensor("WSR", [D2, P], f32r, kind="ExternalInput")
    WMR = nc.dram_tensor("WMR", [D2, P], f32r, kind="ExternalInput")
    W1SN = nc.dram_tensor("W1SN", [D1], f32, kind="ExternalInput")
    W2SN = nc.dram_tensor("W2SN", [D2], f32, kind="ExternalInput")
    B1V = nc.dram_tensor("B1V", [D1], f32, kind="ExternalInput")
    B2V = nc.dram_tensor("B2V", [D2], f32, kind="ExternalInput")
    BSR16 = nc.dram_tensor("BSR16", [P], f32, kind="ExternalInput")
    BS416 = nc.dram_tensor("BS416", [NATT], f32, kind="ExternalInput")
    BMV = nc.dram_tensor("BMV", [P], f32, kind="ExternalInput")
    OUT = nc.dram_tensor("OUT", [NMC, B, P + NATT], f32, kind="ExternalOutput")
    if debug:
        DBG_MU = nc.dram_tensor("DBG_MU", [1, 2 * NBS], f32, kind="ExternalOutput")
        DBG_S = nc.dram_tensor("DBG_S", [8, P], f32, kind="ExternalOutput")
        DBG_PHI = nc.dram_tensor("DBG_PHI", [P, 8, NBS], f32, kind="ExternalOutput")
        DBG_G1 = nc.dram_tensor("DBG_G1", [P, 4, NBS], f32, kind="ExternalOutput")
        DBG_H1 = nc.dram_tensor("DBG_H1", [P, 4, NBS], f32, kind="ExternalOutput")
        DBG_EMB = nc.dram_tensor("DBG_EMB", [P, 2, NBS], f32, kind="ExternalOutput")
        DBG_EN = nc.dram_tensor("DBG_EN", [P, NBS], f32, kind="ExternalOutput")
        DBG_SR = nc.dram_tensor("DBG_SR", [P, NBS], f32, kind="ExternalOutput")
        DBG_Z = nc.dram_tensor("DBG_Z", [P, 4 * NBS], f32, kind="ExternalOutput")

    KT1 = D0 // P    # 8
    KT2 = D1 // P    # 4
    MT1 = RF // P    # 4
    MTH = D1 // P    # 4
    MTE = D2 // P    # 2
    NCH = RPC // P   # 16

    with ExitStack() as ctx:
        tc = ctx.enter_context(tile.TileContext(nc))
        cst = ctx.enter_context(tc.tile_pool(name="cst", bufs=1))
        wp = ctx.enter_context(tc.tile_pool(name="wp", bufs=1))
        omp = ctx.enter_context(tc.tile_pool(name="omp", bufs=1))
        xp = ctx.enter_context(tc.tile_pool(name="xp", bufs=1))
        php = ctx.enter_context(tc.tile_pool(name="php", bufs=2))
        zrp = ctx.enter_context(tc.tile_pool(name="zrp", bufs=1))
        gp = ctx.enter_context(tc.tile_pool(name="gp", bufs=2))
        hp = ctx.enter_context(tc.tile_pool(name="hp", bufs=2))
        sp = ctx.enter_context(tc.tile_pool(name="sp", bufs=2))
        mcp = ctx.enter_context(tc.tile_pool(name="mcp", bufs=1))
        tp = ctx.enter_context(tc.tile_pool(name="tp", bufs=1))
        evp = ctx.enter_context(tc.tile_pool(name="evp", bufs=2))
        zp = ctx.enter_context(tc.tile_pool(name="zp", bufs=1, space="PSUM"))
        pmu = ctx.enter_context(tc.tile_pool(name="pmu", bufs=2, space="PSUM"))
        psc = ctx.enter_context(tc.tile_pool(name="psc", bufs=2, space="PSUM"))
        dram = ctx.enter_context(tc.tile_pool(name="dram", bufs=1, space="DRAM"))

        # ---------- constants & resident weights ----------
        ones_f = cst.tile([P, 1], f32)
        nc.vector.memset(ones_f[:], 1.0 / 1024.0)
        ones = cst.tile([P, 1], f32r)
        nc.vector.tensor_copy(ones[:], ones_f[:])
        halfpi = cst.tile([P, 1], f32)
        nc.vector.memset(halfpi[:], PI / 2)
        ident = cst.tile([P, P], f32)
        make_identity(nc, ident[:])
        ident_r = cst.tile([P, P], f32r)
        nc.vector.tensor_copy(ident_r[:], ident[:])

        w1_sb = wp.tile([P, KT1, D1], f32r, tag="w1")
        w2_sb = wp.tile([P, KT1, D2], f32r, tag="w2")
        for k in range(KT1):
            nc.sync.dma_start(w1_sb[:, k, :], W1T[k * P:(k + 1) * P, :])
            nc.sync.dma_start(w2_sb[:, k, :], W2T[k * P:(k + 1) * P, :])
        ws4_sb = wp.tile([P, MTE, NATT], f32r, tag="ws4")
        wsr_sb = wp.tile([P, MTE, P], f32r, tag="wsr")
        wm_sb = wp.tile([P, MTE, P], f32r, tag="wm")
        for k in range(MTE):
            nc.sync.dma_start(ws4_sb[:, k, :], WS4[k * P:(k + 1) * P, :])
            nc.sync.dma_start(wsr_sb[:, k, :], WSR[k * P:(k + 1) * P, :])
            nc.sync.dma_start(wm_sb[:, k, :], WMR[k * P:(k + 1) * P, :])
        w1sn_sb = wp.tile([P, MTH], f32, tag="w1sn")
        nc.sync.dma_start(w1sn_sb[:], W1SN.rearrange("(t p) -> p t", p=P))
        w2sn_sb = wp.tile([P, MTE], f32, tag="w2sn")
        nc.sync.dma_start(w2sn_sb[:], W2SN.rearrange("(t p) -> p t", p=P))
        b1_sb = wp.tile([P, MTH], f32, tag="b1")
        nc.sync.dma_start(b1_sb[:], B1V.rearrange("(t p) -> p t", p=P))
        b2_sb = wp.tile([P, MTE], f32, tag="b2")
        nc.sync.dma_start(b2_sb[:], B2V.rearrange("(t p) -> p t", p=P))
        bsr_sb = wp.tile([P, 1], f32, tag="bsr")
        nc.sync.dma_start(bsr_sb[:], BSR16[:, None])
        bs4_sb = wp.tile([NATT, 1], f32, tag="bs4")
        nc.sync.dma_start(bs4_sb[:], BS416[:, None])
        bm_sb = wp.tile([P, 1], f32, tag="bm")
        nc.sync.dma_start(bm_sb[:], BMV[:, None])

        st4_spill = dram.tile([NMC, NATT, RPC], f32r, tag="st4_spill")
        en_spill = dram.tile([NMC, P, RPC], f32r, tag="en_spill")
        sr_spill = dram.tile([NMC, P, RPC], f32, tag="sr_spill")

        C_RF = 1.0 / float(np.sqrt(512.0))   # rf-feature scale 1/sqrt(RF)

        def quake_rsqrt(out_ap, v_ap, shp):
            """out = C_RF / sqrt(v): Quake seed + 2 Newton; the final
            iteration's affine constants are pre-scaled by C_RF."""
            h = tp.tile(shp, dt.int32, tag="qk_h")
            nc.vector.tensor_scalar(h[:], v_ap.bitcast(dt.int32), 1, None,
                                    ALU.arith_shift_right)
            nh = tp.tile(shp, dt.int32, tag="qk_nh")
            nc.vector.tensor_tensor(nh[:], h[:], h[:], ALU.bitwise_not)
            yi = tp.tile(shp, dt.int32, tag="qk_yi")
            nc.vector.tensor_scalar(yi[:], nh[:], 0x5F3759DF + 1, None, ALU.add)
            cur = yi[:].bitcast(f32)
            for it in range(2):
                p2 = tp.tile(shp, f32, tag="qk_p2")
                nc.vector.tensor_tensor(p2[:], cur, cur, ALU.mult)
                hh = tp.tile(shp, f32, tag="qk_hh")
                nc.vector.tensor_tensor(hh[:], p2[:], v_ap, ALU.mult)
                g = tp.tile(shp, f32, tag="qk_g")
                cs = C_RF if it == 1 else 1.0
                nc.vector.tensor_scalar(g[:], hh[:], -0.5 * cs, 1.5 * cs,
                                        ALU.mult, ALU.add)
                if it == 1:
                    nc.vector.tensor_tensor(out_ap, cur, g[:], ALU.mult)
                else:
                    yn = tp.tile(shp, f32, tag="qk_yn")
                    nc.vector.tensor_tensor(yn[:], cur, g[:], ALU.mult)
                    cur = yn[:]

        def s_batch(mu_cat, npair, tag):
            """mu_cat sbuf [1, npair*NBS] -> DRAM s/sm flats [1, npair*NBS]."""
            W = npair * NBS
            A = W // P
            d_mu = dram.tile([1, W], f32, tag=f"dmu_{tag}")
            nc.sync.dma_start(d_mu[:], mu_cat[0:1, :W])
            muc = tp.tile([A, P], f32, tag="muc")
            nc.sync.dma_start(muc[:], d_mu[0, :].rearrange("(a b) -> a b", a=A))
            q = tp.tile([A, P], f32, tag="q")
            nc.vector.tensor_tensor(q[:], muc[:], muc[:], ALU.mult)
            v = tp.tile([A, P], f32, tag="v")
            nc.vector.tensor_scalar(v[:], q[:], -1.0 / 512.0, C_VAR, ALU.mult, ALU.add)
            s16 = tp.tile([A, P], f32, tag="s16")
            quake_rsqrt(s16[:], v[:], [A, P])
            sm16 = tp.tile([A, P], f32, tag="sm16")
            nc.vector.tensor_tensor(sm16[:], muc[:], s16[:], ALU.mult)
            d_s = dram.tile([1, W], f32, tag=f"ds_{tag}")
            nc.sync.dma_start(d_s[0, :].rearrange("(a b) -> a b", a=A), s16[:])
            d_sm = dram.tile([1, W], f32, tag=f"dsm_{tag}")
            nc.sync.dma_start(d_sm[0, :].rearrange("(a b) -> a b", a=A), sm16[:])
            return d_s, d_sm

        def layer_front(i, omt, kt, rhs_tile, mu_cat):
            """z^T (feature-major) -> wrap -> cos/sin -> phi [P,8,NBS] f32r;
            mean into mu_cat slice i."""
            zbig = zp.tile([P, MT1 * NBS], f32, tag="zps")
            for mt in range(MT1):
                for k in range(kt):
                    nc.tensor.matmul(
                        zbig[:, mt * NBS:(mt + 1) * NBS],
                        omt[:, k, mt * P:(mt + 1) * P],
                        rhs_tile[:, k, :],
                        start=(k == 0), stop=(k == kt - 1))
            zr = zrp.tile([P, MT1 * NBS], f32, tag="zr")
            if debug and layer_front.dbg_z[0]:
                layer_front.dbg_z[0] = False
                dz = zrp.tile([P, MT1 * NBS], f32, tag="az")
                nc.vector.tensor_copy(dz[:], zbig[:])
                nc.sync.dma_start(DBG_Z[:], dz[:])
            nc.vector.add_range_wrap(zr[:], zbig[:], 0.0, PI, 2 * PI)
            az = zrp.tile([P, MT1 * NBS], f32, tag="az")
            nc.scalar.activation(az[:], zr[:], AF.Abs)
            phi = php.tile([P, 2 * MT1, NBS], f32r, tag="phi")
            flat = phi[:].rearrange("p k n -> p (k n)")
            nc.scalar.activation(flat[:, :MT1 * NBS], az[:], AF.Sin,
                                 bias=halfpi[:], scale=-1.0)
            nc.scalar.activation(flat[:, MT1 * NBS:], zr[:], AF.Sin)
            mu_ps = pmu.tile([1, NBS], f32, tag="mups")
            for k in range(2 * MT1):
                nc.tensor.matmul(mu_ps[:], ones[:], phi[:, k, :],
                                 start=(k == 0), stop=(k == 2 * MT1 - 1))
            nc.scalar.copy(mu_cat[0:1, i * NBS:(i + 1) * NBS], mu_ps[:])
            return phi
        layer_front.dbg_z = [bool(debug)]

        def graw(phi, w_sb, nmt, tagb):
            """G = W^T @ phi, evacuated to SBUF f32: [P, nmt, NBS]."""
            gsb = gp.tile([P, nmt, NBS], f32, tag=f"g_{tagb}")
            for t in range(nmt):
                gps = psc.tile([P, NBS], f32, tag="ps5")
                for k in range(2 * MT1):
                    nc.tensor.matmul(gps[:], w_sb[:, k, t * P:(t + 1) * P],
                                     phi[:, k, :],
                                     start=(k == 0), stop=(k == 2 * MT1 - 1))
                if t % 2 == 0:
                    nc.scalar.copy(gsb[:, t, :], gps[:])
                else:
                    nc.vector.tensor_copy(gsb[:, t, :], gps[:])
            return gsb

        def apply_ln(gsb, nmt, i, d_s, d_sm, wsn_sb, bias_sb, outdt, tagb):
            """out[:,t,:] = s*G + (sm*(-wsum) + b)  (feature-major)."""
            s_b = sp.tile([P, NBS], f32, tag="s_b")
            nc.sync.dma_start(
                s_b[:], d_s[0, i * NBS:(i + 1) * NBS][None, :]
                .to_broadcast((P, NBS)))
            sm_b = sp.tile([P, NBS], f32, tag="sm_b")
            nc.sync.dma_start(
                sm_b[:], d_sm[0, i * NBS:(i + 1) * NBS][None, :]
                .to_broadcast((P, NBS)))
            out = hp.tile([P, nmt, NBS], outdt, tag=f"h_{tagb}")
            for t in range(nmt):
                tmp = tp.tile([P, NBS], f32, tag="ap_tmp")
                nc.gpsimd.tensor_tensor(tmp[:], gsb[:, t, :], s_b[:], ALU.mult)
                nc.vector.affine_then_add(out[:, t, :], sm_b[:], tmp[:],
                                          wsn_sb[:, t:t + 1],
                                          bias_sb[:, t:t + 1])
            return out

        # ================= phase A =================
        for nb in range(NB):
            ncol = slice(nb * NBS, (nb + 1) * NBS)
            xb = xp.tile([P, KT1, NBS], f32r, tag="xb")
            for k in range(KT1):
                nc.sync.dma_start(xb[:, k, :], XT[k * P:(k + 1) * P, ncol])

            for pair in ((0, 1), (2, 3)):
                om1l = {}
                om2l = {}
                for m in pair:
                    o1 = omp.tile([P, KT1, RF], f32r, tag="om1")
                    for k in range(KT1):
                        nc.sync.dma_start(o1[:, k, :],
                                          OM1[m, k * P:(k + 1) * P, :])
                    om1l[m] = o1
                    o2 = omp.tile([P, KT2, RF], f32r, tag="om2")
                    for k in range(KT2):
                        nc.sync.dma_start(o2[:, k, :],
                                          OM2[m, k * P:(k + 1) * P, :])
                    om2l[m] = o2

                mu1_cat = mcp.tile([1, 2 * NBS], f32, tag="mucat")
                g1 = {}
                for i, m in enumerate(pair):
                    phi = layer_front(i, om1l[m], KT1, xb, mu1_cat)
                    if debug and nb == 0 and m == 0:
                        dphi = php.tile([P, 8, NBS], f32, tag="phi")
                        nc.vector.tensor_copy(dphi[:], phi[:])
                        nc.sync.dma_start(DBG_PHI[:], dphi[:])
                    g1[m] = graw(phi, w1_sb, MTH, "1")
                    if debug and nb == 0 and m == 0:
                        nc.sync.dma_start(DBG_G1[:], g1[m][:])
                if debug and nb == 0 and pair[0] == 0:
                    nc.sync.dma_start(DBG_MU[:], mu1_cat[:])
                ds1, dsm1 = s_batch(mu1_cat, 2, f"1_{nb}_{pair[0]}")
                if debug and nb == 0 and pair[0] == 0:
                    dsb = tp.tile([8, P], f32, tag="dbgs")
                    nc.sync.dma_start(dsb[:], ds1[0, :].rearrange("(a b) -> a b", a=8))
                    nc.sync.dma_start(DBG_S[:], dsb[:])

                mu2_cat = mcp.tile([1, 2 * NBS], f32, tag="mucat")
                g2 = {}
                for i, m in enumerate(pair):
                    h1 = apply_ln(g1[m], MTH, i, ds1, dsm1, w1sn_sb, b1_sb,
                                  f32r, "1")
                    if debug and nb == 0 and m == 0:
                        dh1 = php.tile([P, 8, NBS], f32, tag="phi")
                        nc.vector.tensor_copy(dh1[:, :4, :], h1[:])
                        nc.sync.dma_start(DBG_H1[:], dh1[:, :4, :])
                    phi2 = layer_front(i, om2l[m], KT2, h1, mu2_cat)
                    g2[m] = graw(phi2, w2_sb, MTE, "2")
                ds2, dsm2 = s_batch(mu2_cat, 2, f"2_{nb}_{pair[0]}")

                for i, m in enumerate(pair):
                    emb = apply_ln(g2[m], MTE, i, ds2, dsm2, w2sn_sb, b2_sb,
                                   f32r, "2")
                    if debug and nb == 0 and m == 0:
                        demb = php.tile([P, 8, NBS], f32, tag="phi")
                        nc.vector.tensor_copy(demb[:, :2, :], emb[:])
                        nc.sync.dma_start(DBG_EMB[:], demb[:, :2, :])
                    sc4 = psc.tile([P, NBS], f32, tag="ps5")
                    srp = psc.tile([P, NBS], f32, tag="ps5")
                    for k in range(MTE):
                        nc.tensor.matmul(sc4[:NATT, :], ws4_sb[:, k, :],
                                         emb[:, k, :],
                                         start=(k == 0), stop=(k == MTE - 1))
                    for k in range(MTE):
                        nc.tensor.matmul(srp[:], wsr_sb[:, k, :], emb[:, k, :],
                                         start=(k == 0), stop=(k == MTE - 1))
                    sc4sb = evp.tile([NATT, NBS], f32r, tag="sc4sb")
                    nc.scalar.copy(sc4sb[:], sc4[:NATT, :])
                    nc.sync.dma_start(st4_spill[m, :, ncol], sc4sb[:])
                    sr_sb = evp.tile([P, NBS], f32, tag="sr_sb")
                    nc.scalar.copy(sr_sb[:], srp[:])
                    nc.sync.dma_start(sr_spill[m, :, ncol], sr_sb[:])
                    enp = psc.tile([P, NBS], f32, tag="ps5")
                    for k in range(MTE):
                        nc.tensor.matmul(enp[:], wm_sb[:, k, :], emb[:, k, :],
                                         start=(k == 0), stop=(k == MTE - 1))
                    en_sb = evp.tile([P, NBS], f32r, tag="en_sb")
                    nc.vector.tensor_scalar(en_sb[:], enp[:], bm_sb[:], 0.0,
                                            ALU.add, ALU.max)
                    nc.sync.dma_start(en_spill[m, :, ncol], en_sb[:])
                    if debug and nb == 0 and m == 0:
                        den_ = php.tile([P, 8, NBS], f32, tag="phi")
                        nc.vector.tensor_copy(den_[:, 0, :], en_sb[:])
                        nc.vector.tensor_copy(den_[:, 1, :], sr_sb[:])
                        nc.sync.dma_start(DBG_EN[:], den_[:, 0, :])
                        nc.sync.dma_start(DBG_SR[:], den_[:, 1, :])

        # ================= phase B =================
        oh_sb = wp.tile([P, NCH, B], f32r, tag="oh")
        for c in range(NCH):
            nc.sync.dma_start(oh_sb[:, c, :], OH[c * P:(c + 1) * P, :])
        for m in range(NMC):
            e4 = zrp.tile([NATT, RPC], f32r, tag="zr")
            nc.sync.dma_start(e4[:], st4_spill[m])
            nc.scalar.activation(e4[:], e4[:], AF.Exp,
                                 bias=bs4_sb[:], scale=0.0625)
            srm = php.tile([P, RPC], f32, tag="phi")
            nc.sync.dma_start(srm[:], sr_spill[m])
            nc.scalar.activation(srm[:], srm[:], AF.Exp,
                                 bias=bsr_sb[:], scale=0.0625)
            enm = php.tile([P, RPC], f32r, tag="phi")
            nc.sync.dma_start(enm[:], en_spill[m])
            nc.vector.tensor_tensor(enm[:], enm[:], srm[:], ALU.mult)
            segp = psc.tile([P, NBS], f32, tag="ps5")
            seg = segp[:B, :P + NATT]
            for c in range(NCH):
                t1 = zp.tile([P, NBS], f32r, tag="zps")
                nc.tensor.transpose(t1[:, :P], enm[:, c * P:(c + 1) * P],
                                    ident_r[:])
                nc.tensor.transpose(t1[:, P:P + NATT],
                                    e4[:, c * P:(c + 1) * P],
                                    ident_r[:NATT, :NATT])
                vr = evp.tile([P, P + NATT], f32r, tag="vr")
                nc.vector.tensor_copy(vr[:], t1[:, :P + NATT])
                nc.tensor.matmul(seg, oh_sb[:, c, :], vr[:],
                                 start=(c == 0), stop=(c == NCH - 1))
            seg_sb = evp.tile([B, P + NATT], f32, tag="seg_sb")
            nc.vector.tensor_copy(seg_sb[:], seg)
            nc.sync.dma_start(OUT[m], seg_sb[:])

    nc.finalize()
    return nc


def kernel(X, X_idx, Omega1, Omega2, W1, b1, W2, b2, Ws, bs, Wm, bm):
    import sys
    if "/opt/trn_rl_repo" not in sys.path:
        sys.path.insert(0, "/opt/trn_rl_repo")
    from concourse.bass_utils import run_bass_kernel_spmd

    X = np.asarray(X, dtype=np.float32)
    X_idx = np.asarray(X_idx)
    Omega1 = np.asarray(Omega1, dtype=np.float32)
    Omega2 = np.asarray(Omega2, dtype=np.float32)
    W1 = np.asarray(W1, dtype=np.float32)
    W2 = np.asarray(W2, dtype=np.float32)
    Ws = np.asarray(Ws, dtype=np.float32)
    Wm = np.asarray(Wm, dtype=np.float32)
    b1 = np.asarray(b1, dtype=np.float32)
    b2 = np.asarray(b2, dtype=np.float32)
    bs = np.asarray(bs, dtype=np.float32)
    bm = np.asarray(bm, dtype=np.float32)

    if "nc" not in _CACHE:
        _CACHE["nc"] = _build()
    nc = _CACHE["nc"]

    shared = dict(
        OM1=Omega1, OM2=Omega2, W1T=W1, W2T=W2,
        WS4=Ws, WSR=np.ascontiguousarray(np.repeat(Ws, DATT, axis=1)),
        WMR=Wm,
        W1SN=-W1.sum(axis=0), W2SN=-W2.sum(axis=0),
        B1V=b1, B2V=b2,
        BSR16=np.repeat(bs, DATT) / 16.0, BS416=bs / 16.0, BMV=bm,
    )
    in_maps = []
    for c in range(NC):
        rows = slice(c * RPC, (c + 1) * RPC)
        oh = np.zeros((RPC, B), dtype=np.float32)
        oh[np.arange(RPC), X_idx[rows]] = 1.0
        m = dict(shared)
        m["XT"] = np.ascontiguousarray(X[rows].T)
        m["OH"] = oh
        in_maps.append(m)

    res = run_bass_kernel_spmd(nc, in_maps, list(range(NC)))
    num = np.zeros((NMC, B, P), dtype=np.float64)
    den = np.zeros((NMC, B, NATT), dtype=np.float64)
    for c in range(NC):
        out = res.results[c]["OUT"]
        num += out[:, :, :P]
        den += out[:, :, P:]
    emb = num / np.repeat(den, DATT, axis=2)
    return np.ascontiguousarray(emb.transpose(1, 0, 2)).astype(np.float32)
